# revision 14
# baseline (speedup 1.0000x reference)
"""Trainium2 Bass kernel for the 16-level ternary (Haar-style) wavelet
transform of f (len 3^16) with row-orthonormalized 3x3 Phi matrices.

Strategy (v2):
  - Host: QR-orthonormalize the 3x3 Phi blocks; precompute per-level
    2-pass STT coefficients with running scale folding:
      row dot  c0*x0+c1*x1+c2*x2  ==  ((x_a*(ca/cb) + x_b)*(cb/cc) + x_c)*cc
    The trailing *cc of the AVERAGE row is folded into the next level's
    coefficients (and finally into the tail's level-7 matrix), so the
    average branch costs only 2 DVE passes.  Detail rows get their *cc
    applied by the Scalar(ACT) engine, casting fp32 -> bf16 on the way
    out (halves the HBM write traffic; details dominate the output).
  - Engine split per level: DVE = av chain (2 STT) + d1 chain (2 STT);
    GPSIMD = d2 chain (2 STT); ACT = the two scale-casts.  This takes
    DVE from 6 to 4 passes/level; all four engines land at ~85-95us.
  - Main SPMD kernel (8 cores): f is split into contiguous chunks
    aligned to units of 3^7 = 2187 elements; each unit recurses levels
    0..6 inside one SBUF partition.  Details DMA straight out per level
    as bf16.  Levels 3..6 batch all tiles in resident SBUF buffers.
  - Tail: per-level f7 (3^9 elems) is AllGathered and levels 7..15 run
    redundantly on every core (fp32, exact).
"""

import sys

for _p in ("/opt/trn_rl_repo",):
    if _p not in sys.path:
        sys.path.append(_p)

import numpy as np

import concourse.bass as bass
import concourse.mybir as mybir
import concourse.tile as tile
from concourse.bass_utils import run_bass_kernel_spmd

F32 = mybir.dt.float32
BF16 = mybir.dt.bfloat16
MULT = mybir.AluOpType.mult
ADD = mybir.AluOpType.add

NL = 16                   # total levels
LK = 7                    # levels computed by the main kernel (0..6)
UNIT = 3 ** LK            # 2187 input elems per unit
NUNITS = 3 ** (NL - LK)   # 19683 units overall
NCORES = 8
UPP = 2                   # units per partition per tile
T = 10                    # tiles per core
PAD_UNITS = T * 128 * UPP  # 2560 padded units per core

# contiguous unit ranges per core (2461 x7 + 2456)
_base = [0]
for _k in range(NCORES):
    _base.append(_base[-1] + (2461 if _k < 7 else NUNITS - 7 * 2461))
CORE_U0 = _base[:-1]
CORE_UN = [_base[k + 1] - _base[k] for k in range(NCORES)]

# main-kernel output layout (per core, in bf16 elements)
OFF_D1 = []
OFF_D2 = []
_off = 0
for _i in range(LK):
    _w = 3 ** (6 - _i)
    OFF_D1.append(_off)
    OFF_D2.append(_off + PAD_UNITS * _w)
    _off += 2 * PAD_UNITS * _w
OUT_LEN = _off

# cf tensor layout: per level, 8 floats:
# [av_g1, av_g2, d1_g1, d1_g2, d1_s, d2_g1, d2_g2, d2_s]
CFW = 8


def _phi_from_inputs(Phi_P: np.ndarray) -> np.ndarray:
    Q = np.stack([np.linalg.qr(Phi_P[i].T.astype(np.float32))[0]
                  for i in range(Phi_P.shape[0])])
    return np.transpose(Q, (0, 2, 1)).astype(np.float32)


def _prep_coeffs(Phi: np.ndarray):
    """Per-level 2-pass coefficients + permutations + running scale.

    Returns (cf [LK,8] fp32, perms [LK][3] of (a,b,c), s7)."""
    cf = np.zeros((LK, CFW), np.float32)
    perms = []
    s = np.float64(1.0)
    for l in range(LK):
        M = Phi[l].astype(np.float64) * s
        rowperms = []
        for r in range(3):
            c = M[r]
            a, b, cc = np.argsort(np.abs(c))
            rowperms.append((int(a), int(b), int(cc)))
            base = 0 if r == 0 else (2 if r == 1 else 5)
            if r == 2:
                # d2 runs as 2 ACT muls + 2 GPSIMD adds + ACT scale:
                #   d2 = (xa*(g1*g2) + xb*g2 + xc) * c_c
                cf[l, base + 0] = (c[a] / c[b]) * (c[b] / c[cc])
                cf[l, base + 1] = c[b] / c[cc]
                cf[l, base + 2] = c[cc]
            else:
                cf[l, base + 0] = c[a] / c[b]
                cf[l, base + 1] = c[b] / c[cc]
                if r > 0:
                    cf[l, base + 2] = c[cc]
        perms.append(rowperms)
        s = M[0][rowperms[0][2]]      # av trailing scale folds forward
    return cf, perms, np.float32(s)


def _split_multi_waits(nc):
    """This walrus build rejects any instruction carrying >1 sync wait
    ("Too many sync wait commands").  Split extra waits onto single-wait
    NOPs inserted just before, on the same engine queue (queue order makes
    the semantics identical)."""
    ctr = [0]
    for fn in nc.m.functions:
        for bb in fn.blocks:
            new = []
            for inst in bb.instructions:
                si = inst.sync_info
                if si is not None and si.on_wait and len(si.on_wait) > 1:
                    waits = list(si.on_wait)
                    for w in waits[:-1]:
                        ctr[0] += 1
                        new.append(mybir.InstNoOp(
                            name=f"splitw_{ctr[0]}",
                            engine=inst.engine,
                            bass_nofuse=True,
                            sync_info=mybir.SyncInfo(on_wait=[w], on_update=[]),
                        ))
                    si.on_wait = [waits[-1]]
                new.append(inst)
            bb.instructions = new


def _triple(nc, dst, src, phi_sb, prow, pcol0):
    """dst[p, r] = sum_j src[p, 3r+j] * phi_sb[p, pcol0+j] (tail helper)."""
    W = src.shape[-1]
    Wo = W // 3
    x0 = src[:, 0::3]
    x1 = src[:, 1::3]
    x2 = src[:, 2::3]
    c0 = phi_sb[:, pcol0 + 0: pcol0 + 1]
    c1 = phi_sb[:, pcol0 + 1: pcol0 + 2]
    c2 = phi_sb[:, pcol0 + 2: pcol0 + 3]
    assert x0.shape[-1] == Wo and dst.shape[-1] == Wo
    nc.vector.tensor_scalar_mul(dst, x0, c0)
    nc.vector.scalar_tensor_tensor(dst, x1, c1, dst, MULT, ADD)
    nc.vector.scalar_tensor_tensor(dst, x2, c2, dst, MULT, ADD)


def _emit_tail(nc, pool, phi_sb, f7_tensor, f7_off, tail_out):
    """Levels 7..15 on the gathered f7 (19683 elems), all in SBUF."""
    X = pool.tile([81, 243], F32, tag="X7", name="X7")
    nc.sync.dma_start(X[:], bass.AP(f7_tensor, f7_off, [[243, 81], [1, 243]]))
    cur = X[:]
    for L in range(LK, 12):                    # levels 7..11 on [81, W]
        Wo = cur.shape[-1] // 3
        d1 = pool.tile([81, Wo], F32, tag=f"td1_{L}", name=f"td1_{L}")
        d2 = pool.tile([81, Wo], F32, tag=f"td2_{L}", name=f"td2_{L}")
        av = pool.tile([81, Wo], F32, tag=f"ta_{L}", name=f"ta_{L}")
        pc = L * 9
        _triple(nc, av[:], cur, phi_sb[0:81, :], 0, pc + 0)
        _triple(nc, d1[:], cur, phi_sb[0:81, :], 1, pc + 3)
        _triple(nc, d2[:], cur, phi_sb[0:81, :], 2, pc + 6)
        base = 3 ** (15 - L)
        for dt_, off in ((d1, base), (d2, 2 * base)):
            nc.sync.dma_start(
                bass.AP(tail_out, off, [[Wo, 81], [1, Wo]]), dt_[:])
        cur = av[:]
    A12T = pool.tile([1, 81], F32, tag="A12T", name="A12T")
    nc.sync.dma_start(A12T[:], cur)
    cur = A12T[:]
    for L in range(12, NL):                    # levels 12..15 on [1, W]
        Wo = cur.shape[-1] // 3
        d1 = pool.tile([1, Wo], F32, tag=f"td1_{L}", name=f"td1_{L}")
        d2 = pool.tile([1, Wo], F32, tag=f"td2_{L}", name=f"td2_{L}")
        av = pool.tile([1, Wo], F32, tag=f"ta_{L}", name=f"ta_{L}")
        pc = L * 9
        _triple(nc, av[:], cur, phi_sb[0:1, :], 0, pc + 0)
        _triple(nc, d1[:], cur, phi_sb[0:1, :], 1, pc + 3)
        _triple(nc, d2[:], cur, phi_sb[0:1, :], 2, pc + 6)
        base = 3 ** (15 - L)
        for dt_, off in ((d1, base), (d2, 2 * base)):
            nc.sync.dma_start(
                bass.AP(tail_out, off, [[Wo, 1], [1, Wo]]), dt_[:])
        cur = av[:]
    nc.sync.dma_start(bass.AP(tail_out, 0, [[1, 1], [1, 1]]), cur)


def build_main(perms, nrep=1, in_bufs=2, merge_tail=True):
    nc = bass.Bass("TRN2", target_bir_lowering=False, debug=False,
                   num_devices=NCORES)
    x = nc.dram_tensor("x", [PAD_UNITS * UNIT], F32, kind="ExternalInput")
    cf = nc.dram_tensor("cf", [128, LK * CFW], F32, kind="ExternalInput")
    phi = nc.dram_tensor("phi", [128, NL * 9], F32, kind="ExternalInput")
    out = nc.dram_tensor("out", [OUT_LEN], BF16, kind="ExternalOutput")
    tail_out = (nc.dram_tensor("tail", [NUNITS], F32, kind="ExternalOutput")
                if merge_tail else None)

    FW = UPP * UNIT  # 4374 elems per partition per tile

    with tile.TileContext(nc) as tc:
        with (
            tc.tile_pool(name="cf_p", bufs=1) as cf_pool,
            tc.tile_pool(name="phi_p", bufs=1) as phi_pool,
            tc.tile_pool(name="in_p", bufs=in_bufs) as in_pool,
            tc.tile_pool(name="a_p", bufs=2) as a_pool,
            tc.tile_pool(name="u_p", bufs=2) as u_pool,
            tc.tile_pool(name="w_p", bufs=2) as w_pool,
            tc.tile_pool(name="d_p", bufs=2) as d_pool,
            tc.tile_pool(name="r_p", bufs=1) as r_pool,
            tc.tile_pool(name="dd_p", bufs=1) as dd_pool,
            tc.tile_pool(name="dram_p", bufs=1, space="DRAM") as dram_pool,
            tc.tile_pool(name="tail_p", bufs=1) as tail_pool,
        ):
            def chain(dst, src, lvl, row, tmp=None):
                """2-pass DVE STT row dot (unscaled):
                dst = (sum_j c_j x_j)/c_c."""
                a, b, c = perms[lvl][row]
                base = 0 if row == 0 else 2
                col = lvl * CFW + base
                g1 = cf_sb[:, col + 0: col + 1]
                g2 = cf_sb[:, col + 1: col + 2]
                t = tmp if tmp is not None else dst
                nc.vector.scalar_tensor_tensor(
                    t, src[:, a::3], g1, src[:, b::3], MULT, ADD)
                nc.vector.scalar_tensor_tensor(
                    dst, t, g2, src[:, c::3], MULT, ADD)

            def d2_adds(pool, src, lvl, Wo, tag):
                """d2 pre-scale sum: 2 ACT muls + 2 GPSIMD adds.
                Returns the fp32 sum tile (scale c_c still pending)."""
                a, b, c = perms[lvl][2]
                col = lvl * CFW + 5
                ma = pool.tile([128, Wo], F32, tag=f"ma{tag}",
                               name=f"ma{tag}")
                mb = pool.tile([128, Wo], F32, tag=f"mb{tag}",
                               name=f"mb{tag}")
                nc.scalar.mul(ma[:], src[:, a::3], cf_sb[:, col: col + 1])
                nc.scalar.mul(mb[:], src[:, b::3],
                              cf_sb[:, col + 1: col + 2])
                t = pool.tile([128, Wo], F32, tag=f"t2{tag}",
                              name=f"t2{tag}")
                nc.gpsimd.tensor_tensor(t[:], ma[:], mb[:], ADD)
                nc.gpsimd.tensor_tensor(t[:], t[:], src[:, c::3], ADD)
                return t

            def scale_out(d_bf, src_f32, lvl, row):
                col = lvl * CFW + (4 if row == 1 else 7)
                nc.scalar.mul(d_bf, src_f32, cf_sb[:, col: col + 1])

            def body():
                global cf_sb
                cf_sb = cf_pool.tile([128, LK * CFW], F32, tag="cf",
                                     name="cf_sb")
                nc.sync.dma_start(cf_sb[:], cf[:])
                phi_sb = phi_pool.tile([128, NL * 9], F32, tag="phi",
                                       name="phi_sb")
                nc.sync.dma_start(phi_sb[:], phi[:])

                # resident buffers for levels 3..6 (+ f7 slab)
                R = {3: r_pool.tile([128, T * UPP * 81], F32, tag="R3",
                                    name="R3")}
                for lvl in range(4, 7):
                    R[lvl] = r_pool.tile(
                        [128, T * UPP * 3 ** (7 - lvl)], F32,
                        tag=f"R{lvl}", name=f"R{lvl}")
                F7 = r_pool.tile([128, T * UPP], F32, tag="F7", name="F7")

                # ---- streamed levels 0..2, one [128, UPP*2187] tile each
                for t in range(T):
                    xt = in_pool.tile([128, FW], F32, tag="xt", name="xt")
                    src = bass.AP(x, t * 128 * FW, [[FW, 128], [1, FW]])
                    nc.sync.dma_start(xt[:], src)

                    cur = xt[:]
                    pend = []
                    for lvl in range(3):
                        w = 3 ** (6 - lvl)
                        Wo = cur.shape[-1] // 3
                        if lvl < 2:
                            av = a_pool.tile([128, Wo], F32, tag=f"a{lvl}",
                                             name=f"a{lvl}")
                            av_ap = av[:]
                        else:
                            av_ap = R[3][:, t * UPP * 81:(t + 1) * UPP * 81]
                        tav = u_pool.tile([128, Wo], F32, tag=f"tav{lvl}",
                                          name=f"tav{lvl}")
                        chain(av_ap, cur, lvl, 0, tmp=tav[:])
                        u = u_pool.tile([128, Wo], F32, tag=f"u{lvl}",
                                        name=f"u{lvl}")
                        chain(u[:], cur, lvl, 1)
                        wd = d2_adds(w_pool, cur, lvl, Wo, f"s{lvl}")
                        pend.append((lvl, w, u, wd))
                        cur = av_ap
                    # deferred scale-casts + output DMAs (keeps the ACT
                    # queue's muls ahead of its GPS/DVE-dependent scales)
                    for lvl, w, u, wd in pend:
                        d1 = d_pool.tile([128, UPP * w], BF16,
                                         tag=f"d1_{lvl}", name=f"d1_{lvl}")
                        d2 = d_pool.tile([128, UPP * w], BF16,
                                         tag=f"d2_{lvl}", name=f"d2_{lvl}")
                        scale_out(d1[:], u[:], lvl, 1)
                        scale_out(d2[:], wd[:], lvl, 2)
                        uw = UPP * w
                        for dt_, off in ((d1, OFF_D1[lvl]), (d2, OFF_D2[lvl])):
                            dst = bass.AP(out, off + t * 128 * uw,
                                          [[uw, 128], [1, uw]])
                            nc.sync.dma_start(dst, dt_[:])

                # ---- batched levels 3..6 over the whole resident buffer
                for lvl in range(3, LK):
                    w = 3 ** (6 - lvl)
                    cur = R[lvl][:]
                    Wo = cur.shape[-1] // 3
                    av_ap = R[lvl + 1][:] if lvl < 6 else F7[:]
                    tav = dd_pool.tile([128, Wo], F32, tag=f"btav{lvl}",
                                       name=f"btav{lvl}")
                    chain(av_ap, cur, lvl, 0, tmp=tav[:])
                    u = dd_pool.tile([128, Wo], F32, tag=f"bu{lvl}",
                                     name=f"bu{lvl}")
                    chain(u[:], cur, lvl, 1)
                    wd = d2_adds(dd_pool, cur, lvl, Wo, f"b{lvl}")
                    d1 = dd_pool.tile([128, Wo], BF16, tag=f"bd1_{lvl}",
                                      name=f"bd1_{lvl}")
                    d2 = dd_pool.tile([128, Wo], BF16, tag=f"bd2_{lvl}",
                                      name=f"bd2_{lvl}")
                    scale_out(d1[:], u[:], lvl, 1)
                    scale_out(d2[:], wd[:], lvl, 2)
                    uw = UPP * w
                    for dt_, off in ((d1, OFF_D1[lvl]), (d2, OFF_D2[lvl])):
                        dst = bass.AP(out, off,
                                      [[uw, 128], [128 * uw, T], [1, uw]])
                        src3 = dt_[:].rearrange("p (t c) -> p t c", t=T)
                        nc.sync.dma_start(dst, src3)

                # f7 slab to DRAM in unit order: unit g = t*256 + p*2 + j
                f7_loc = dram_pool.tile([PAD_UNITS], F32, tag="f7_loc",
                                        name="f7_loc")
                nc.sync.dma_start(
                    bass.AP(f7_loc.tensor, f7_loc[:].offset,
                            [[UPP, 128], [128 * UPP, T], [1, UPP]]),
                    F7[:].rearrange("p (t j) -> p t j", t=T))

                if merge_tail:
                    f7_all = dram_pool.tile([NCORES * PAD_UNITS], F32,
                                            tag="f7_all", name="f7_all",
                                            addr_space="Shared")
                    nc.gpsimd.collective_compute(
                        "AllGather",
                        mybir.AluOpType.bypass,
                        replica_groups=[list(range(NCORES))],
                        ins=[f7_loc.opt()],
                        outs=[f7_all.opt()],
                    )
                    f7_flat = dram_pool.tile([NUNITS], F32, tag="f7_flat",
                                             name="f7_flat")
                    for j in range(NCORES):
                        nc.sync.dma_start(
                            f7_flat[CORE_U0[j]:CORE_U0[j] + CORE_UN[j]],
                            f7_all[j * PAD_UNITS:j * PAD_UNITS + CORE_UN[j]])
                    _emit_tail(nc, tail_pool, phi_sb, f7_flat.tensor,
                               f7_flat[:].offset, tail_out)

            if nrep == 1:
                body()
            else:
                with tc.For_i(0, nrep, 1):
                    body()

    return nc


def build_tail(nrep=1):
    nc = bass.Bass("TRN2", target_bir_lowering=False, debug=False,
                   num_devices=1)
    f7 = nc.dram_tensor("f7", [NUNITS], F32, kind="ExternalInput")
    phi = nc.dram_tensor("phi2", [128, NL * 9], F32, kind="ExternalInput")
    outt = nc.dram_tensor("tail", [NUNITS], F32, kind="ExternalOutput")

    with tile.TileContext(nc) as tc:
        with (
            tc.tile_pool(name="phi_p", bufs=1) as phi_pool,
            tc.tile_pool(name="w_p", bufs=1) as wp,
        ):
            def body():
                phi_sb = phi_pool.tile([128, NL * 9], F32, tag="phi")
                nc.sync.dma_start(phi_sb[:], phi[:])
                _emit_tail(nc, wp, phi_sb, f7, 0, outt)

            if nrep == 1:
                body()
            else:
                with tc.For_i(0, nrep, 1):
                    body()

    return nc


_CACHE = {}

MERGED_TAIL = True


def _host_inputs(f: np.ndarray, Phi_P: np.ndarray):
    Phi = _phi_from_inputs(np.asarray(Phi_P, dtype=np.float32))
    cf, perms, s7 = _prep_coeffs(Phi)
    phi_t = Phi.copy()
    phi_t[LK] = phi_t[LK] * s7       # tail level-7 matrix absorbs the fold
    cf_all = np.broadcast_to(cf.reshape(1, LK * CFW),
                             (128, LK * CFW)).copy()
    phi_all = np.broadcast_to(phi_t.reshape(1, NL * 9), (128, NL * 9)).copy()
    return cf_all, phi_all, perms, cf


def kernel(f: np.ndarray, Phi_P: np.ndarray) -> np.ndarray:
    f = np.asarray(f, dtype=np.float32).ravel()
    cf_all, phi_all, perms, cf = _host_inputs(f, Phi_P)

    # The slice permutations are structural (baked into APs at build), so
    # the cached kernel is only valid while they match.
    if "main" not in _CACHE or _CACHE["perms"] != perms:
        _CACHE["main"] = build_main(perms, merge_tail=MERGED_TAIL)
        _split_multi_waits(_CACHE["main"])
        _CACHE["perms"] = perms
    nc_main = _CACHE["main"]

    in_maps = []
    for k in range(NCORES):
        lo = CORE_U0[k] * UNIT
        n = CORE_UN[k] * UNIT
        xk = np.zeros(PAD_UNITS * UNIT, dtype=np.float32)
        xk[:n] = f[lo:lo + n]
        in_maps.append({"x": xk, "cf": cf_all, "phi": phi_all})

    res = run_bass_kernel_spmd(nc_main, in_maps, list(range(NCORES)))

    f_hat = np.empty(3 ** NL, dtype=np.float32)
    for k in range(NCORES):
        ok = res.results[k]["out"]
        u0, un = CORE_U0[k], CORE_UN[k]
        for i in range(LK):
            w = 3 ** (6 - i)
            base = 3 ** (15 - i)
            f_hat[base + u0 * w: base + (u0 + un) * w] = \
                ok[OFF_D1[i]: OFF_D1[i] + un * w].astype(np.float32)
            f_hat[2 * base + u0 * w: 2 * base + (u0 + un) * w] = \
                ok[OFF_D2[i]: OFF_D2[i] + un * w].astype(np.float32)

    f_hat[:NUNITS] = res.results[0]["tail"]
    return f_hat


# revision 20
# speedup vs baseline: 1.0285x; 1.0285x over previous
"""Trainium2 Bass kernel for the 16-level ternary (Haar-style) wavelet
transform of f (len 3^16) with row-orthonormalized 3x3 Phi matrices.

Strategy (v2):
  - Host: QR-orthonormalize the 3x3 Phi blocks; precompute per-level
    2-pass STT coefficients with running scale folding:
      row dot  c0*x0+c1*x1+c2*x2  ==  ((x_a*(ca/cb) + x_b)*(cb/cc) + x_c)*cc
    The trailing *cc of the AVERAGE row is folded into the next level's
    coefficients (and finally into the tail's level-7 matrix), so the
    average branch costs only 2 DVE passes.  Detail rows get their *cc
    applied by the Scalar(ACT) engine, casting fp32 -> bf16 on the way
    out (halves the HBM write traffic; details dominate the output).
  - Engine split per level: DVE = av chain (2 STT) + d1 chain (2 STT);
    GPSIMD = d2 chain (2 STT); ACT = the two scale-casts.  This takes
    DVE from 6 to 4 passes/level; all four engines land at ~85-95us.
  - Main SPMD kernel (8 cores): f is split into contiguous chunks
    aligned to units of 3^7 = 2187 elements; each unit recurses levels
    0..6 inside one SBUF partition.  Details DMA straight out per level
    as bf16.  Levels 3..6 batch all tiles in resident SBUF buffers.
  - Tail: per-level f7 (3^9 elems) is AllGathered and levels 7..15 run
    redundantly on every core (fp32, exact).
"""

import sys

for _p in ("/opt/trn_rl_repo",):
    if _p not in sys.path:
        sys.path.append(_p)

import numpy as np

import concourse.bass as bass
import concourse.mybir as mybir
import concourse.tile as tile
from concourse.bass_utils import run_bass_kernel_spmd

F32 = mybir.dt.float32
BF16 = mybir.dt.bfloat16
MULT = mybir.AluOpType.mult
ADD = mybir.AluOpType.add

NL = 16                   # total levels
LK = 7                    # levels computed by the main kernel (0..6)
UNIT = 3 ** LK            # 2187 input elems per unit
NUNITS = 3 ** (NL - LK)   # 19683 units overall
NCORES = 8
UPP = 2                   # units per partition per tile
T = 10                    # tiles per core
PAD_UNITS = T * 128 * UPP  # 2560 padded units per core

# contiguous unit ranges per core (2461 x7 + 2456)
_base = [0]
for _k in range(NCORES):
    _base.append(_base[-1] + (2461 if _k < 7 else NUNITS - 7 * 2461))
CORE_U0 = _base[:-1]
CORE_UN = [_base[k + 1] - _base[k] for k in range(NCORES)]

# main-kernel output layout (per core, in bf16 elements)
OFF_D1 = []
OFF_D2 = []
_off = 0
for _i in range(LK):
    _w = 3 ** (6 - _i)
    OFF_D1.append(_off)
    OFF_D2.append(_off + PAD_UNITS * _w)
    _off += 2 * PAD_UNITS * _w
OUT_LEN = _off

# cf tensor layout: per level, 8 floats:
# [av_g1, av_g2, d1_g1, d1_g2, d1_s, d2_g1, d2_g2, d2_s]
CFW = 8


def _phi_from_inputs(Phi_P: np.ndarray) -> np.ndarray:
    Q = np.stack([np.linalg.qr(Phi_P[i].T.astype(np.float32))[0]
                  for i in range(Phi_P.shape[0])])
    return np.transpose(Q, (0, 2, 1)).astype(np.float32)


def _prep_coeffs(Phi: np.ndarray):
    """Per-level 2-pass coefficients + permutations + running scale.

    Returns (cf [LK,8] fp32, perms [LK][3] of (a,b,c), s7)."""
    cf = np.zeros((LK, CFW), np.float32)
    perms = []
    s = np.float64(1.0)
    for l in range(LK):
        M = Phi[l].astype(np.float64) * s
        rowperms = []
        for r in range(2):
            c = M[r]
            a, b, cc = np.argsort(np.abs(c))
            rowperms.append((int(a), int(b), int(cc)))
            base = 0 if r == 0 else 2
            cf[l, base + 0] = c[a] / c[b]
            cf[l, base + 1] = c[b] / c[cc]
            if r > 0:
                cf[l, base + 2] = c[cc]
        # d2 via the orthonormal reconstruction identity, reusing the
        # (dense) av' and d1u chains:
        #   d2'' = x_c2 + av'*(-A) + d1u*(-B);  d2 = d2''*C
        c0 = rowperms[0][2]
        c1 = rowperms[1][2]
        c2 = int(np.argmax(np.abs(Phi[l, 2, :])))
        rowperms.append((c2, c2, c2))
        cf[l, 5] = -(Phi[l, 0, c0] * Phi[l, 0, c2])
        cf[l, 6] = -(Phi[l, 1, c1] * Phi[l, 1, c2])
        cf[l, 7] = s / Phi[l, 2, c2]
        perms.append(rowperms)
        s = M[0][rowperms[0][2]]      # av trailing scale folds forward
    return cf, perms, np.float32(s)


def _split_multi_waits(nc):
    """This walrus build rejects any instruction carrying >1 sync wait
    ("Too many sync wait commands").  Split extra waits onto single-wait
    NOPs inserted just before, on the same engine queue (queue order makes
    the semantics identical)."""
    ctr = [0]
    for fn in nc.m.functions:
        for bb in fn.blocks:
            new = []
            for inst in bb.instructions:
                si = inst.sync_info
                if si is not None and si.on_wait and len(si.on_wait) > 1:
                    waits = list(si.on_wait)
                    for w in waits[:-1]:
                        ctr[0] += 1
                        new.append(mybir.InstNoOp(
                            name=f"splitw_{ctr[0]}",
                            engine=inst.engine,
                            bass_nofuse=True,
                            sync_info=mybir.SyncInfo(on_wait=[w], on_update=[]),
                        ))
                    si.on_wait = [waits[-1]]
                new.append(inst)
            bb.instructions = new


def _triple(nc, dst, src, phi_sb, prow, pcol0):
    """dst[p, r] = sum_j src[p, 3r+j] * phi_sb[p, pcol0+j] (tail helper)."""
    W = src.shape[-1]
    Wo = W // 3
    x0 = src[:, 0::3]
    x1 = src[:, 1::3]
    x2 = src[:, 2::3]
    c0 = phi_sb[:, pcol0 + 0: pcol0 + 1]
    c1 = phi_sb[:, pcol0 + 1: pcol0 + 2]
    c2 = phi_sb[:, pcol0 + 2: pcol0 + 3]
    assert x0.shape[-1] == Wo and dst.shape[-1] == Wo
    nc.vector.tensor_scalar_mul(dst, x0, c0)
    nc.vector.scalar_tensor_tensor(dst, x1, c1, dst, MULT, ADD)
    nc.vector.scalar_tensor_tensor(dst, x2, c2, dst, MULT, ADD)


def _emit_tail(nc, pool, phi_sb, f7_tensor, f7_off, tail_out):
    """Levels 7..15 on the gathered f7 (19683 elems), all in SBUF."""
    X = pool.tile([81, 243], F32, tag="X7", name="X7")
    nc.sync.dma_start(X[:], bass.AP(f7_tensor, f7_off, [[243, 81], [1, 243]]))
    cur = X[:]
    for L in range(LK, 12):                    # levels 7..11 on [81, W]
        Wo = cur.shape[-1] // 3
        d1 = pool.tile([81, Wo], F32, tag=f"td1_{L}", name=f"td1_{L}")
        d2 = pool.tile([81, Wo], F32, tag=f"td2_{L}", name=f"td2_{L}")
        av = pool.tile([81, Wo], F32, tag=f"ta_{L}", name=f"ta_{L}")
        pc = L * 9
        _triple(nc, av[:], cur, phi_sb[0:81, :], 0, pc + 0)
        _triple(nc, d1[:], cur, phi_sb[0:81, :], 1, pc + 3)
        _triple(nc, d2[:], cur, phi_sb[0:81, :], 2, pc + 6)
        base = 3 ** (15 - L)
        for dt_, off in ((d1, base), (d2, 2 * base)):
            nc.sync.dma_start(
                bass.AP(tail_out, off, [[Wo, 81], [1, Wo]]), dt_[:])
        cur = av[:]
    A12T = pool.tile([1, 81], F32, tag="A12T", name="A12T")
    nc.sync.dma_start(A12T[:], cur)
    cur = A12T[:]
    for L in range(12, NL):                    # levels 12..15 on [1, W]
        Wo = cur.shape[-1] // 3
        d1 = pool.tile([1, Wo], F32, tag=f"td1_{L}", name=f"td1_{L}")
        d2 = pool.tile([1, Wo], F32, tag=f"td2_{L}", name=f"td2_{L}")
        av = pool.tile([1, Wo], F32, tag=f"ta_{L}", name=f"ta_{L}")
        pc = L * 9
        _triple(nc, av[:], cur, phi_sb[0:1, :], 0, pc + 0)
        _triple(nc, d1[:], cur, phi_sb[0:1, :], 1, pc + 3)
        _triple(nc, d2[:], cur, phi_sb[0:1, :], 2, pc + 6)
        base = 3 ** (15 - L)
        for dt_, off in ((d1, base), (d2, 2 * base)):
            nc.sync.dma_start(
                bass.AP(tail_out, off, [[Wo, 1], [1, Wo]]), dt_[:])
        cur = av[:]
    nc.sync.dma_start(bass.AP(tail_out, 0, [[1, 1], [1, 1]]), cur)


def build_main(perms, nrep=1, in_bufs=2, merge_tail=True, d2_mode="gps"):
    nc = bass.Bass("TRN2", target_bir_lowering=False, debug=False,
                   num_devices=NCORES)
    x = nc.dram_tensor("x", [PAD_UNITS * UNIT], F32, kind="ExternalInput")
    cf = nc.dram_tensor("cf", [128, LK * CFW], F32, kind="ExternalInput")
    phi = nc.dram_tensor("phi", [128, NL * 9], F32, kind="ExternalInput")
    out = nc.dram_tensor("out", [OUT_LEN], BF16, kind="ExternalOutput")
    tail_out = (nc.dram_tensor("tail", [NUNITS], F32, kind="ExternalOutput")
                if merge_tail else None)

    FW = UPP * UNIT  # 4374 elems per partition per tile

    with tile.TileContext(nc) as tc:
        with (
            tc.tile_pool(name="cf_p", bufs=1) as cf_pool,
            tc.tile_pool(name="phi_p", bufs=1) as phi_pool,
            tc.tile_pool(name="in_p", bufs=in_bufs) as in_pool,
            tc.tile_pool(name="a_p", bufs=2) as a_pool,
            tc.tile_pool(name="u_p", bufs=2) as u_pool,
            tc.tile_pool(name="w_p", bufs=2) as w_pool,
            tc.tile_pool(name="d_p", bufs=2) as d_pool,
            tc.tile_pool(name="r_p", bufs=1) as r_pool,
            tc.tile_pool(name="dd_p", bufs=1) as dd_pool,
            tc.tile_pool(name="dram_p", bufs=1, space="DRAM") as dram_pool,
            tc.tile_pool(name="tail_p", bufs=1) as tail_pool,
        ):
            def chain(dst, src, lvl, row, tmp=None):
                """2-pass DVE STT row dot (unscaled):
                dst = (sum_j c_j x_j)/c_c."""
                a, b, c = perms[lvl][row]
                base = 0 if row == 0 else 2
                col = lvl * CFW + base
                g1 = cf_sb[:, col + 0: col + 1]
                g2 = cf_sb[:, col + 1: col + 2]
                t = tmp if tmp is not None else dst
                nc.vector.scalar_tensor_tensor(
                    t, src[:, a::3], g1, src[:, b::3], MULT, ADD)
                nc.vector.scalar_tensor_tensor(
                    dst, t, g2, src[:, c::3], MULT, ADD)

            def d2_ident(pool, src, lvl, Wo, tag, avp, d1u):
                """d2 pre-scale via the reconstruction identity:
                  t = x_c2 + avp*(-A) + d1u*(-B)
                ACT: 2 dense muls; GPSIMD: 2 adds (one strided x read).
                d2_mode="dve" runs both adds as DVE STTs instead."""
                c2 = perms[lvl][2][2]
                col = lvl * CFW + 5
                t = pool.tile([128, Wo], F32, tag=f"t2{tag}",
                              name=f"t2{tag}")
                if d2_mode == "dve":
                    nc.vector.scalar_tensor_tensor(
                        t[:], avp, cf_sb[:, col: col + 1],
                        src[:, c2::3], MULT, ADD)
                    nc.vector.scalar_tensor_tensor(
                        t[:], d1u, cf_sb[:, col + 1: col + 2],
                        t[:], MULT, ADD)
                    return t
                ma = pool.tile([128, Wo], F32, tag=f"ma{tag}",
                               name=f"ma{tag}")
                mb = pool.tile([128, Wo], F32, tag=f"mb{tag}",
                               name=f"mb{tag}")
                nc.scalar.mul(ma[:], avp, cf_sb[:, col: col + 1])
                nc.scalar.mul(mb[:], d1u, cf_sb[:, col + 1: col + 2])
                nc.gpsimd.tensor_tensor(t[:], src[:, c2::3], ma[:], ADD)
                nc.gpsimd.tensor_tensor(t[:], t[:], mb[:], ADD)
                return t

            def scale_out(d_bf, src_f32, lvl, row):
                col = lvl * CFW + (4 if row == 1 else 7)
                nc.scalar.mul(d_bf, src_f32, cf_sb[:, col: col + 1])

            def body():
                global cf_sb
                cf_sb = cf_pool.tile([128, LK * CFW], F32, tag="cf",
                                     name="cf_sb")
                nc.sync.dma_start(cf_sb[:], cf[:])
                phi_sb = phi_pool.tile([128, NL * 9], F32, tag="phi",
                                       name="phi_sb")
                nc.sync.dma_start(phi_sb[:], phi[:])

                # resident buffers for levels 3..6 (+ f7 slab)
                R = {3: r_pool.tile([128, T * UPP * 81], F32, tag="R3",
                                    name="R3")}
                for lvl in range(4, 7):
                    R[lvl] = r_pool.tile(
                        [128, T * UPP * 3 ** (7 - lvl)], F32,
                        tag=f"R{lvl}", name=f"R{lvl}")
                F7 = r_pool.tile([128, T * UPP], F32, tag="F7", name="F7")

                # ---- streamed levels 0..2, one [128, UPP*2187] tile each
                for t in range(T):
                    xt = in_pool.tile([128, FW], F32, tag="xt", name="xt")
                    src = bass.AP(x, t * 128 * FW, [[FW, 128], [1, FW]])
                    nc.sync.dma_start(xt[:], src)

                    cur = xt[:]
                    pend = []
                    for lvl in range(3):
                        w = 3 ** (6 - lvl)
                        Wo = cur.shape[-1] // 3
                        if lvl < 2:
                            av = a_pool.tile([128, Wo], F32, tag=f"a{lvl}",
                                             name=f"a{lvl}")
                            av_ap = av[:]
                        else:
                            av_ap = R[3][:, t * UPP * 81:(t + 1) * UPP * 81]
                        tav = u_pool.tile([128, Wo], F32, tag=f"tav{lvl}",
                                          name=f"tav{lvl}")
                        chain(av_ap, cur, lvl, 0, tmp=tav[:])
                        u = u_pool.tile([128, Wo], F32, tag=f"u{lvl}",
                                        name=f"u{lvl}")
                        chain(u[:], cur, lvl, 1)
                        wd = d2_ident(w_pool, cur, lvl, Wo, f"s{lvl}",
                                      av_ap, u[:])
                        pend.append((lvl, w, u, wd))
                        cur = av_ap
                    # deferred scale-casts + output DMAs (keeps the ACT
                    # queue's muls ahead of its GPS/DVE-dependent scales)
                    for lvl, w, u, wd in pend:
                        d1 = d_pool.tile([128, UPP * w], BF16,
                                         tag=f"d1_{lvl}", name=f"d1_{lvl}")
                        d2 = d_pool.tile([128, UPP * w], BF16,
                                         tag=f"d2_{lvl}", name=f"d2_{lvl}")
                        scale_out(d1[:], u[:], lvl, 1)
                        scale_out(d2[:], wd[:], lvl, 2)
                        uw = UPP * w
                        for dt_, off in ((d1, OFF_D1[lvl]), (d2, OFF_D2[lvl])):
                            dst = bass.AP(out, off + t * 128 * uw,
                                          [[uw, 128], [1, uw]])
                            nc.sync.dma_start(dst, dt_[:])

                # ---- batched levels 3..6 over the whole resident buffer
                for lvl in range(3, LK):
                    w = 3 ** (6 - lvl)
                    cur = R[lvl][:]
                    Wo = cur.shape[-1] // 3
                    av_ap = R[lvl + 1][:] if lvl < 6 else F7[:]
                    tav = dd_pool.tile([128, Wo], F32, tag=f"btav{lvl}",
                                       name=f"btav{lvl}")
                    chain(av_ap, cur, lvl, 0, tmp=tav[:])
                    u = dd_pool.tile([128, Wo], F32, tag=f"bu{lvl}",
                                     name=f"bu{lvl}")
                    chain(u[:], cur, lvl, 1)
                    wd = d2_ident(dd_pool, cur, lvl, Wo, f"b{lvl}",
                                  av_ap, u[:])
                    d1 = dd_pool.tile([128, Wo], BF16, tag=f"bd1_{lvl}",
                                      name=f"bd1_{lvl}")
                    d2 = dd_pool.tile([128, Wo], BF16, tag=f"bd2_{lvl}",
                                      name=f"bd2_{lvl}")
                    scale_out(d1[:], u[:], lvl, 1)
                    scale_out(d2[:], wd[:], lvl, 2)
                    uw = UPP * w
                    for dt_, off in ((d1, OFF_D1[lvl]), (d2, OFF_D2[lvl])):
                        dst = bass.AP(out, off,
                                      [[uw, 128], [128 * uw, T], [1, uw]])
                        src3 = dt_[:].rearrange("p (t c) -> p t c", t=T)
                        nc.sync.dma_start(dst, src3)

                # f7 slab to DRAM in unit order: unit g = t*256 + p*2 + j
                f7_loc = dram_pool.tile([PAD_UNITS], F32, tag="f7_loc",
                                        name="f7_loc")
                nc.sync.dma_start(
                    bass.AP(f7_loc.tensor, f7_loc[:].offset,
                            [[UPP, 128], [128 * UPP, T], [1, UPP]]),
                    F7[:].rearrange("p (t j) -> p t j", t=T))

                if merge_tail:
                    f7_all = dram_pool.tile([NCORES * PAD_UNITS], F32,
                                            tag="f7_all", name="f7_all",
                                            addr_space="Shared")
                    nc.gpsimd.collective_compute(
                        "AllGather",
                        mybir.AluOpType.bypass,
                        replica_groups=[list(range(NCORES))],
                        ins=[f7_loc.opt()],
                        outs=[f7_all.opt()],
                    )
                    f7_flat = dram_pool.tile([NUNITS], F32, tag="f7_flat",
                                             name="f7_flat")
                    for j in range(NCORES):
                        nc.sync.dma_start(
                            f7_flat[CORE_U0[j]:CORE_U0[j] + CORE_UN[j]],
                            f7_all[j * PAD_UNITS:j * PAD_UNITS + CORE_UN[j]])
                    _emit_tail(nc, tail_pool, phi_sb, f7_flat.tensor,
                               f7_flat[:].offset, tail_out)

            if nrep == 1:
                body()
            else:
                with tc.For_i(0, nrep, 1):
                    body()

    return nc


def build_tail(nrep=1):
    nc = bass.Bass("TRN2", target_bir_lowering=False, debug=False,
                   num_devices=1)
    f7 = nc.dram_tensor("f7", [NUNITS], F32, kind="ExternalInput")
    phi = nc.dram_tensor("phi2", [128, NL * 9], F32, kind="ExternalInput")
    outt = nc.dram_tensor("tail", [NUNITS], F32, kind="ExternalOutput")

    with tile.TileContext(nc) as tc:
        with (
            tc.tile_pool(name="phi_p", bufs=1) as phi_pool,
            tc.tile_pool(name="w_p", bufs=1) as wp,
        ):
            def body():
                phi_sb = phi_pool.tile([128, NL * 9], F32, tag="phi")
                nc.sync.dma_start(phi_sb[:], phi[:])
                _emit_tail(nc, wp, phi_sb, f7, 0, outt)

            if nrep == 1:
                body()
            else:
                with tc.For_i(0, nrep, 1):
                    body()

    return nc


_CACHE = {}

MERGED_TAIL = True


def _host_inputs(f: np.ndarray, Phi_P: np.ndarray):
    Phi = _phi_from_inputs(np.asarray(Phi_P, dtype=np.float32))
    cf, perms, s7 = _prep_coeffs(Phi)
    phi_t = Phi.copy()
    phi_t[LK] = phi_t[LK] * s7       # tail level-7 matrix absorbs the fold
    cf_all = np.broadcast_to(cf.reshape(1, LK * CFW),
                             (128, LK * CFW)).copy()
    phi_all = np.broadcast_to(phi_t.reshape(1, NL * 9), (128, NL * 9)).copy()
    return cf_all, phi_all, perms, cf


def kernel(f: np.ndarray, Phi_P: np.ndarray) -> np.ndarray:
    f = np.asarray(f, dtype=np.float32).ravel()
    cf_all, phi_all, perms, cf = _host_inputs(f, Phi_P)

    # The slice permutations are structural (baked into APs at build), so
    # the cached kernel is only valid while they match.
    if "main" not in _CACHE or _CACHE["perms"] != perms:
        _CACHE["main"] = build_main(perms, merge_tail=MERGED_TAIL)
        _split_multi_waits(_CACHE["main"])
        _CACHE["perms"] = perms
    nc_main = _CACHE["main"]

    in_maps = []
    for k in range(NCORES):
        lo = CORE_U0[k] * UNIT
        n = CORE_UN[k] * UNIT
        xk = np.zeros(PAD_UNITS * UNIT, dtype=np.float32)
        xk[:n] = f[lo:lo + n]
        in_maps.append({"x": xk, "cf": cf_all, "phi": phi_all})

    res = run_bass_kernel_spmd(nc_main, in_maps, list(range(NCORES)))

    f_hat = np.empty(3 ** NL, dtype=np.float32)
    for k in range(NCORES):
        ok = res.results[k]["out"]
        u0, un = CORE_U0[k], CORE_UN[k]
        for i in range(LK):
            w = 3 ** (6 - i)
            base = 3 ** (15 - i)
            f_hat[base + u0 * w: base + (u0 + un) * w] = \
                ok[OFF_D1[i]: OFF_D1[i] + un * w].astype(np.float32)
            f_hat[2 * base + u0 * w: 2 * base + (u0 + un) * w] = \
                ok[OFF_D2[i]: OFF_D2[i] + un * w].astype(np.float32)

    f_hat[:NUNITS] = res.results[0]["tail"]
    return f_hat
